# revision 16
# baseline (speedup 1.0000x reference)
"""Trainium2 Bass kernel for MultiHeadSyntonicAttention.

Problem: B=2, S=2048, D=1024, H=16 heads, DH=64.
  q/k/v = Linear(query/key/value); per-head gnosis gate
  gn = sigmoid(k . wg + bg); scores = (q k^T / sqrt(dh)) * (1+gn);
  out = softmax(scores) v;  out = ((out Wo+bo) Wd+bd) Wh+bh.

Sharding (8 cores): core c -> batch b=c//4, head-group g=c%4 (4 heads).
Each core computes its heads' attention and a row-slice partial of the
fused output projection Wf = Wo@Wd@Wh; host sums 4 partials per batch.

Device layout (everything "transposed", tokens on the free axis):
  QT[cb][128, S] bf16 (2 heads' dh on partitions) = Wq_s^T qT
  ST[k=128, q=1024] per (a-half, head, kb) f32 in PSUM
  pt = exp(ST * gsc) bf16  (per-partition scale; max-sub skipped)
  ot[128, 1024] accum over kb: lhsT=[V_h | ones] -> rows 64:128 = denom
  ctxT = otc[0:64] * recip(otc[64:128]) -> out = ctxT^T Wf (+host bf)

Schedule: one continuous ACT(exp)-paced software pipeline across all 8
(a-half, head) passes.  Each head's last PV / PSUM-drain copy / the
reciprocal+mult normalize-finish are carried into the NEXT heads' kb
iterations as scheduled pops, so no PE matmul ever queues behind a
DVE-FIFO bubble and the next head's QK/exp start immediately at each
boundary.  Head 0 additionally defers PV(kb>=8) into head 1 (head 0's
iterations carry the 16 V-projections).  Half-0's output projection +
DMA-out run during half-1's attention; the final normalize runs on the
then-idle ACT engine as exp(-ln(den)) feeding a chunked tail.
PSUM: st0/st1 double-buffered score tiles + aux0/aux1 alternating PV
accumulators; background psum borrows the idle aux tag.  Weights ship
packed [128, 2048] (4KB DMA rows; 512B-row transfers run ~4x slower).
"""

import sys

sys.path.insert(0, "/opt/trn_rl_repo")

import numpy as np
import ml_dtypes

BF16 = ml_dtypes.bfloat16
WSCALE = 1.0   # fp8 shipping fails the 2e-2 bar (softmax ctx is sqrt(N)-
               # suppressed; elementwise noise passes through at full size)

B, S, D, H = 2, 2048, 1024, 16
DH = D // H          # 64
HPC = 4              # heads per core
C = HPC * DH         # 256 head-local columns per core
NCORES = 8
ND = D // 128        # 8 d-chunks
NSB = S // 128       # 16 s-blocks
AW = 1024            # attention q-tile width (q-half)
NA = S // AW         # 2

_nc_cache = {}


def build_bass():
    import concourse.bass as bass
    import concourse.mybir as mybir
    import concourse.tile as tile
    from concourse import bacc

    f32 = mybir.dt.float32
    bf16 = mybir.dt.bfloat16
    Alu = mybir.AluOpType
    Act = mybir.ActivationFunctionType

    nc = bacc.Bacc(None, target_bir_lowering=False, name="syntonic_attn")

    qT_d = nc.dram_tensor("qT", [D, S], bf16, kind="ExternalInput")
    kT_d = nc.dram_tensor("kT", [D, S], bf16, kind="ExternalInput")
    vT_d = nc.dram_tensor("vT", [D, S], bf16, kind="ExternalInput")
    wq_d = nc.dram_tensor("wq", [128, ND * C], bf16, kind="ExternalInput")
    wk_d = nc.dram_tensor("wk", [128, ND * C], bf16, kind="ExternalInput")
    wv_d = nc.dram_tensor("wv", [128, ND * C], bf16, kind="ExternalInput")
    wf_d = nc.dram_tensor("wf", [C, D], bf16, kind="ExternalInput")
    wg4_d = nc.dram_tensor("wg4", [C, HPC], bf16, kind="ExternalInput")
    bq_d = nc.dram_tensor("bq", [1, C], bf16, kind="ExternalInput")
    bk_d = nc.dram_tensor("bk", [1, C], bf16, kind="ExternalInput")
    bv_d = nc.dram_tensor("bv", [1, C], bf16, kind="ExternalInput")
    bg_d = nc.dram_tensor("bg128", [128, 1], f32, kind="ExternalInput")
    out_d = nc.dram_tensor("out", [S, D], bf16, kind="ExternalOutput")

    GSC = 0.125 / (WSCALE * WSCALE)

    with tile.TileContext(nc) as tc:
        with (
            tc.tile_pool(name="res", bufs=1) as res,
            tc.tile_pool(name="acts", bufs=1) as acts,
            tc.tile_pool(name="work", bufs=2) as work,
            tc.tile_pool(name="outp", bufs=4) as outp,
            tc.tile_pool(name="psum", bufs=1, space="PSUM") as psum,
        ):
            # ---------------- resident input tiles ----------------
            qT = [res.tile([128, S], bf16, tag=f"qT{i}", name=f"qT{i}") for i in range(ND)]
            kT = [res.tile([128, S], bf16, tag=f"kT{i}", name=f"kT{i}") for i in range(ND)]
            vT = [res.tile([128, S], bf16, tag=f"vT{i}", name=f"vT{i}") for i in range(ND)]
            wqp = res.tile([128, ND * C], bf16, tag="wqp")
            wkp = res.tile([128, ND * C], bf16, tag="wkp")
            wvp = res.tile([128, ND * C], bf16, tag="wvp")
            wf = [res.tile([128, D], bf16, tag=f"wf{i}", name=f"wf{i}") for i in range(2)]
            wg4 = [res.tile([128, HPC], bf16, tag=f"wg4{i}", name=f"wg4{i}")
                   for i in range(2)]
            bq = res.tile([1, C], bf16, tag="bq")
            bk = res.tile([1, C], bf16, tag="bk")
            bv = res.tile([1, C], bf16, tag="bv")
            bg128 = res.tile([128, 1], f32, tag="bg128")
            ones = res.tile([1, 512], bf16, tag="ones")

            def wsl(wt, dc, cb):
                return wt[:, dc * C + cb * 128: dc * C + (cb + 1) * 128]

            # ---------------- DMA emission (arrival order = priority) ------
            nc.sync.dma_start(wkp[:], wk_d[:, :])
            nc.sync.dma_start(bk[:], bk_d[:])
            nc.sync.dma_start(bg128[:], bg_d[:])
            for i in range(2):
                nc.sync.dma_start(wg4[i][:], wg4_d[i * 128:(i + 1) * 128, :])
            for cols in (slice(0, 1024), slice(1024, 2048)):   # kT in halves
                for i in range(ND):
                    nc.sync.dma_start(kT[i][:, cols], kT_d[i * 128:(i + 1) * 128, cols])
            nc.sync.dma_start(wqp[:], wq_d[:, :])
            nc.sync.dma_start(bq[:], bq_d[:])
            for i in range(ND):   # qT first half (Q proj a=0)
                nc.sync.dma_start(qT[i][:, 0:1024], qT_d[i * 128:(i + 1) * 128, 0:1024])
            nc.sync.dma_start(wvp[:], wv_d[:, :])
            nc.sync.dma_start(bv[:], bv_d[:])
            for cols in (slice(0, 1024), slice(1024, 2048)):
                for i in range(ND):
                    nc.sync.dma_start(vT[i][:, cols], vT_d[i * 128:(i + 1) * 128, cols])
            for i in range(ND):   # qT second half
                nc.sync.dma_start(qT[i][:, 1024:2048],
                                  qT_d[i * 128:(i + 1) * 128, 1024:2048])
            for i in range(2):
                nc.sync.dma_start(wf[i][:], wf_d[i * 128:(i + 1) * 128, :])
            nc.vector.memset(ones[:], 1.0)

            # ---------------- persistent activation tiles ----------------
            QT = [acts.tile([128, S], bf16, tag=f"QT{i}", name=f"QT{i}") for i in range(2)]
            KT = [acts.tile([128, S], bf16, tag=f"KT{i}", name=f"KT{i}") for i in range(2)]
            ctxT = [acts.tile([128, S], bf16, tag=f"ctxT{i}", name=f"ctxT{i}")
                    for i in range(2)]
            gscT = acts.tile([128, NSB * HPC], f32, tag="gscT")
            V = [acts.tile([128, HPC * 2 * DH], bf16, tag=f"V{i}", name=f"V{i}")
                 for i in range(NSB)]

            st_alt = [0]

            def project_chunk(wt, x_tiles, bias, dest, cb, qs, pname,
                              tag=None, skip_gc=False):
                # one 512-wide output chunk dest[:, qs] (+ rank-1 bias)
                if tag is None:
                    tag = f"st{st_alt[0] % 2}"
                    st_alt[0] += 1
                ps = psum.tile([128, 512], f32, tag=tag, name=pname)
                for dc in range(ND):
                    nc.tensor.matmul(
                        ps[:], wsl(wt, dc, cb), x_tiles[dc][:, qs],
                        start=(dc == 0), stop=False, skip_group_check=skip_gc,
                    )
                nc.tensor.matmul(
                    ps[:], bias[0:1, cb * 128:(cb + 1) * 128], ones[0:1, 0:512],
                    start=False, stop=True, skip_group_check=skip_gc,
                )
                nc.vector.tensor_copy(dest[:, qs], ps[:])

            def emit_vproj(sb, tag):
                nc.vector.memset(V[sb][:], 1.0)
                ps = psum.tile([128, C], f32, tag=tag, name=f"psv{sb}")
                for dc in range(ND):
                    nc.tensor.matmul(
                        ps[:], vT[dc][:, sb * 128:(sb + 1) * 128],
                        wvp[:, dc * C:(dc + 1) * C],
                        start=(dc == 0), stop=False, skip_group_check=True,
                    )
                nc.tensor.matmul(ps[:], ones[0:1, 0:128], bv[:], start=False,
                                 stop=True, skip_group_check=True)
                nc.vector.tensor_copy(
                    V[sb][:].rearrange("p (h x) -> p h x", h=HPC)[:, :, 0:DH],
                    ps[:].rearrange("p (h x) -> p h x", h=HPC),
                )

            def emit_outproj(a, qb, tag, cast_engine=None):
                # out rows [a*1024 + qb*128 : +128]; contract ctx dims (2 cb)
                ps = psum.tile([128, 1024], f32, tag=tag, name=f"pso{a}{qb}")
                row0 = a * 1024 + qb * 128
                for oc in range(2):
                    for cc in range(2):
                        nc.tensor.matmul(
                            ps[:, oc * 512:(oc + 1) * 512],
                            ctxT[cc][:, row0:row0 + 128],
                            wf[cc][:, oc * 512:(oc + 1) * 512],
                            start=(cc == 0), stop=(cc == 1),
                            skip_group_check=True,
                        )
                ob = outp.tile([128, D], bf16, tag="ob", name=f"ob{a}{qb}")
                if cast_engine == "scalar":
                    nc.scalar.activation(ob[:], ps[:], Act.Copy)
                else:
                    nc.vector.tensor_copy(ob[:], ps[:])
                nc.sync.dma_start(out_d[row0:row0 + 128, :], ob[:])

            # ---------------- prephase: K proj + gates pipelined with DMA --
            gps = psum.tile([128, NSB * HPC], f32, tag="aux0", name="gps")
            gn = work.tile([128, NSB * HPC], f32, tag="gn", bufs=1)
            for t in range(4):
                ts_ = slice(t * 512, (t + 1) * 512)
                for cb in range(2):
                    project_chunk(wkp, kT, bk, KT[cb], cb, ts_, f"psk{cb}{t}")
                for sb in range(4 * t, 4 * t + 4):
                    for cc in range(2):
                        nc.tensor.matmul(
                            gps[:, sb * HPC:(sb + 1) * HPC],
                            KT[cc][:, sb * 128:(sb + 1) * 128],
                            wg4[cc][:],
                            start=(cc == 0), stop=(cc == 1),
                        )
                gsl = slice(4 * t * HPC, (4 * t + 4) * HPC)
                nc.scalar.activation(gn[:, gsl], gps[:, gsl], Act.Sigmoid,
                                     bias=bg128[:], scale=1.0)
                nc.vector.tensor_scalar(gscT[:, gsl], gn[:, gsl], GSC, GSC,
                                        Alu.mult, Alu.add)

            # Q proj for cb0, a=0 (first head's rhs)
            for t in range(2):
                project_chunk(wqp, qT, bq, QT[0], 0, slice(t * 512, (t + 1) * 512),
                              f"psq00{t}")

            # ---------------- pop schedule ----------------
            pops = {}

            def add_pop(si, kb, fn):
                pops.setdefault((si, kb), []).append(fn)

            # V proj during head 0 (its PV(kb>=8) are deferred to head 1)
            for sb in range(NSB - 1):
                add_pop(0, sb + 1, (lambda sb=sb: emit_vproj(sb, "aux1")))
            # Q proj cb1 a0 (needed at si=2): aux0 freed by otc(0,0) @ (1,kb9)
            for t in range(2):
                add_pop(1, 11 + 2 * t,
                        (lambda t=t: project_chunk(
                            wqp, qT, bq, QT[1], 1, slice(t * 512, (t + 1) * 512),
                            f"psq10{t}", tag="aux0", skip_gc=True)))
            # Q proj a1 (needed at si=4/si=6): aux1 freed by otc(0,1) @ (2,kb1)
            for t in range(2):
                add_pop(2, 3 + 2 * t,
                        (lambda t=t: project_chunk(
                            wqp, qT, bq, QT[0], 0,
                            slice(1024 + t * 512, 1024 + (t + 1) * 512),
                            f"psq01{t}", tag="aux1", skip_gc=True)))
            for t in range(2):
                add_pop(2, 7 + 2 * t,
                        (lambda t=t: project_chunk(
                            wqp, qT, bq, QT[1], 1,
                            slice(1024 + t * 512, 1024 + (t + 1) * 512),
                            f"psq11{t}", tag="aux1", skip_gc=True)))
            # out-proj of half 0: after normfin(0,3) completes ~ (4,kb8)
            for j in range(4):
                add_pop(4, 9 + 2 * j, (lambda j=j: emit_outproj(0, j, "aux1")))
            for j in range(4, 8):
                add_pop(5, 3 + 2 * (j - 4),
                        (lambda j=j: emit_outproj(0, j, "aux0")))

            # ---------------- attention: continuous cross-head pipeline ----
            seq = [(a, h) for a in range(NA) for h in range(HPC)]
            # normfin(si) pop slots: (target_si, [kb_c0, kb_c1])
            NF_SLOT = {0: (2, (2, 4)), 1: (3, (2, 4)), 2: (3, (6, 8)),
                       3: (4, (2, 4)), 4: (6, (2, 4)), 5: (7, (2, 4)),
                       6: (7, (6, 8))}
            carry = []   # closures from the previous head, one per early iter

            for si, (a, h) in enumerate(seq):
                qs = slice(a * AW, (a + 1) * AW)
                cb, po = h // 2, (h % 2) * 64
                vsl = slice(h * 2 * DH, (h + 1) * 2 * DH)
                ot = psum.tile([128, AW], f32, tag=f"aux{h % 2}",
                               name=f"ot{a}{h}")
                defer = 8 if si == 0 else 15   # PV(kb>=defer) carried onward
                pts = {}

                def mk_pv(j, ot=ot, vsl=vsl, pts=pts, defer=defer):
                    def f():
                        ptp = pts.pop(j)
                        for qc in range(2):
                            cs = slice(qc * 512, (qc + 1) * 512)
                            nc.tensor.matmul(
                                ot[:, cs], V[j][:, vsl], ptp[:, cs],
                                start=(j == 0), stop=(j == NSB - 1),
                                skip_group_check=True)
                    return f

                for kb in range(NSB):
                    kslc = slice(kb * 128, (kb + 1) * 128)
                    st = psum.tile([128, AW], f32, tag=f"st{kb % 2}",
                                   name=f"st{a}{h}{kb}")
                    for qc in range(2):
                        cs = slice(qc * 512, (qc + 1) * 512)
                        aqs = slice(a * AW + qc * 512, a * AW + (qc + 1) * 512)
                        nc.tensor.matmul(st[:, cs], KT[cb][po:po + 64, kslc],
                                         QT[cb][po:po + 64, aqs],
                                         start=True, stop=True,
                                         skip_group_check=True)
                    pt = work.tile([128, AW], bf16, tag="pt",
                                   name=f"pt{a}{h}{kb}", bufs=10)
                    nc.scalar.activation(pt[:], st[:], Act.Exp,
                                         scale=gscT[:, kb * HPC + h:kb * HPC + h + 1])
                    pts[kb] = pt
                    if carry:
                        carry.pop(0)()
                    for fn in pops.get((si, kb), ()):
                        fn()
                    if 1 <= kb and kb - 1 < defer:
                        mk_pv(kb - 1)()

                # hand the rest of this head to the next heads' iterations
                newcarry = [mk_pv(j) for j in range(defer, NSB)]
                if si == 0:
                    newcarry.insert(0, lambda: emit_vproj(NSB - 1, "aux1"))

                def mk_otc(si=si, a=a, h=h, cb=cb, po=po, ot=ot):
                    otc = work.tile([128, AW], f32, tag="otc",
                                    name=f"otc{a}{h}", bufs=2)
                    nc.vector.tensor_copy(otc[:], ot[:])

                    def norm_fin_chunk(part):
                        rec = work.tile([DH, 512], f32, tag=f"rec{part}",
                                        name=f"rec{a}{h}{part}", bufs=2)
                        cs = slice(part * 512, (part + 1) * 512)
                        qcs = slice(a * AW + part * 512, a * AW + (part + 1) * 512)
                        nc.vector.reciprocal(rec[:], otc[DH:2 * DH, cs])
                        nc.vector.tensor_tensor(
                            ctxT[cb][po:po + 64, qcs], otc[0:DH, cs],
                            rec[:], Alu.mult)

                    if si in NF_SLOT:
                        tsi, kbs = NF_SLOT[si]
                        add_pop(tsi, kbs[0], lambda: norm_fin_chunk(0))
                        add_pop(tsi, kbs[1], lambda: norm_fin_chunk(1))
                        return None
                    return otc

                if si + 1 < len(seq):
                    newcarry.append(mk_otc)
                    carry = newcarry
                else:
                    # tail: drain the last PV, then normalize on the idle ACT
                    # engine: exp(-ln(den)), chunked mult + out-proj of half 1.
                    for c in newcarry:
                        c()
                    otcf = mk_otc()
                    lt = work.tile([DH, AW], f32, tag="ltf", name="lt", bufs=1)
                    rec = work.tile([DH, AW], f32, tag="recf", name="recf", bufs=1)
                    nc.scalar.activation(lt[:], otcf[DH:2 * DH, :], Act.Ln)
                    nc.scalar.activation(rec[:], lt[:], Act.Exp, scale=-1.0)
                    for cchunk in range(4):
                        cs = slice(cchunk * 256, (cchunk + 1) * 256)
                        qcs = slice(a * AW + cchunk * 256,
                                    a * AW + (cchunk + 1) * 256)
                        nc.vector.tensor_tensor(
                            ctxT[cb][po:po + 64, qcs], otcf[0:DH, cs],
                            rec[:, cs], Alu.mult)
                        for qb in (2 * cchunk, 2 * cchunk + 1):
                            emit_outproj(1, qb, f"aux{qb % 2}",
                                         cast_engine=("scalar", None)[qb % 2])

    nc.finalize()
    return nc


def get_nc():
    if "nc" not in _nc_cache:
        _nc_cache["nc"] = build_bass()
    return _nc_cache["nc"]


def pack_w(W, cols):
    """[D, C] weight slice -> packed [128, ND*C] (dc-major along free)."""
    Wc = np.ascontiguousarray(np.asarray(W, np.float32)[:, cols])
    return np.concatenate([Wc[dc * 128:(dc + 1) * 128, :] for dc in range(ND)],
                          axis=1).astype(BF16)


def make_in_maps(query, key_, value, Wq, bq, Wk, bk, Wv, bv, wg, bg, Wo, bo, Wd, bd, Wh, bh):
    """Host-side sharding: returns (in_maps for 8 cores, fused bias)."""
    f = np.asarray
    Wf = f(Wo, np.float64) @ f(Wd, np.float64) @ f(Wh, np.float64)
    bf = (f(bo, np.float64) @ f(Wd, np.float64) @ f(Wh, np.float64)
          + f(bd, np.float64) @ f(Wh, np.float64) + f(bh, np.float64))

    wg4 = np.zeros((C, HPC), np.float32)
    for h in range(HPC):
        wg4[h * DH:(h + 1) * DH, h] = np.asarray(wg, np.float32)
    wg4 = wg4.astype(BF16)
    bg128 = np.full((128, 1), np.float32(bg), np.float32)

    xT = []
    for b in range(B):
        xT.append(tuple(
            np.ascontiguousarray(np.asarray(x[b], np.float32).T).astype(BF16)
            for x in (query, key_, value)
        ))

    in_maps = []
    for c in range(NCORES):
        b, g = divmod(c, HPC)
        cols = slice(g * C, (g + 1) * C)
        qTb, kTb, vTb = xT[b]
        in_maps.append({
            "qT": qTb, "kT": kTb, "vT": vTb,
            "wq": pack_w(Wq, cols), "wk": pack_w(Wk, cols), "wv": pack_w(Wv, cols),
            "wf": np.ascontiguousarray(Wf[cols, :]).astype(BF16),
            "wg4": wg4, "bg128": bg128,
            "bq": np.asarray(bq, np.float32)[None, cols].astype(BF16),
            "bk": np.asarray(bk, np.float32)[None, cols].astype(BF16),
            "bv": np.asarray(bv, np.float32)[None, cols].astype(BF16),
        })
    return in_maps, bf.astype(np.float32)


def gather(results, bf):
    out = np.zeros((B, S, D), np.float32)
    for c in range(NCORES):
        b = c // HPC
        out[b] += np.asarray(results[c]["out"], np.float32)
    out += bf[None, None, :]
    return out


def kernel(**inputs):
    from concourse.bass_utils import run_bass_kernel_spmd

    nc = get_nc()
    in_maps, bf = make_in_maps(**inputs)
    res = run_bass_kernel_spmd(nc, in_maps, core_ids=list(range(NCORES)))
    return gather(res.results, bf)


# revision 17
# speedup vs baseline: 1.1541x; 1.1541x over previous
"""Trainium2 Bass kernel for MultiHeadSyntonicAttention.

Problem: B=2, S=2048, D=1024, H=16 heads, DH=64.
  q/k/v = Linear(query/key/value); per-head gnosis gate
  gn = sigmoid(k . wg + bg); scores = (q k^T / sqrt(dh)) * (1+gn);
  out = softmax(scores) v;  out = ((out Wo+bo) Wd+bd) Wh+bh.

Sharding (8 cores): core c -> batch b=c//4, head-group g=c%4 (4 heads).
Each core computes its heads' attention and a row-slice partial of the
fused output projection Wf = Wo@Wd@Wh; host sums 4 partials per batch.

Device layout (everything "transposed", tokens on the free axis):
  QT[cb][128, S] bf16 (2 heads' dh on partitions) = Wq_s^T qT
  ST[k=128, q=1024] per (a-half, head, kb) f32 in PSUM
  pt = exp(ST * gsc) bf16  (per-partition scale; max-sub skipped)
  ot[128, 1024] accum over kb: lhsT=[V_h | ones] -> rows 64:128 = denom
  ctxT = otc[0:64] * recip(otc[64:128]) -> out = ctxT^T Wf (+host bf)

Schedule: one continuous ACT(exp)-paced software pipeline across all 8
(a-half, head) passes.  Each head's last PV / PSUM-drain copy / the
reciprocal+mult normalize-finish are carried into the NEXT heads' kb
iterations as scheduled pops, so no PE matmul ever queues behind a
DVE-FIFO bubble and the next head's QK/exp start immediately at each
boundary.  Head 0 additionally defers PV(kb>=8) into head 1 (head 0's
iterations carry the 16 V-projections).  Half-0's output projection +
DMA-out run during half-1's attention; the final normalize runs on the
then-idle ACT engine as exp(-ln(den)) feeding a chunked tail.
PSUM: st0/st1 double-buffered score tiles + aux0/aux1 alternating PV
accumulators; background psum borrows the idle aux tag.  Weights ship
packed [128, 2048] (4KB DMA rows; 512B-row transfers run ~4x slower).
"""

import sys

sys.path.insert(0, "/opt/trn_rl_repo")

import numpy as np
import ml_dtypes

BF16 = ml_dtypes.bfloat16
WSCALE = 1.0   # fp8 shipping fails the 2e-2 bar (softmax ctx is sqrt(N)-
               # suppressed; elementwise noise passes through at full size)

B, S, D, H = 2, 2048, 1024, 16
DH = D // H          # 64
HPC = 4              # heads per core
C = HPC * DH         # 256 head-local columns per core
NCORES = 8
ND = D // 128        # 8 d-chunks
NSB = S // 128       # 16 s-blocks
AW = 1024            # attention q-tile width (q-half)
NA = S // AW         # 2

_nc_cache = {}


def build_bass():
    import concourse.bass as bass
    import concourse.mybir as mybir
    import concourse.tile as tile
    from concourse import bacc

    f32 = mybir.dt.float32
    bf16 = mybir.dt.bfloat16
    Alu = mybir.AluOpType
    Act = mybir.ActivationFunctionType

    nc = bacc.Bacc(None, target_bir_lowering=False, name="syntonic_attn")

    qT_d = nc.dram_tensor("qT", [D, S], bf16, kind="ExternalInput")
    kT_d = nc.dram_tensor("kT", [D, S], bf16, kind="ExternalInput")
    vT_d = nc.dram_tensor("vT", [D, S], bf16, kind="ExternalInput")
    wq_d = nc.dram_tensor("wq", [128, ND * C], bf16, kind="ExternalInput")
    wk_d = nc.dram_tensor("wk", [128, ND * C], bf16, kind="ExternalInput")
    wv_d = nc.dram_tensor("wv", [128, ND * C], bf16, kind="ExternalInput")
    wf_d = nc.dram_tensor("wf", [C, D], bf16, kind="ExternalInput")
    wg4_d = nc.dram_tensor("wg4", [C, HPC], bf16, kind="ExternalInput")
    bq_d = nc.dram_tensor("bq", [1, C], bf16, kind="ExternalInput")
    bk_d = nc.dram_tensor("bk", [1, C], bf16, kind="ExternalInput")
    bv_d = nc.dram_tensor("bv", [1, C], bf16, kind="ExternalInput")
    bg_d = nc.dram_tensor("bg128", [128, 1], f32, kind="ExternalInput")
    out_d = nc.dram_tensor("out", [S, D], bf16, kind="ExternalOutput")

    GSC = 0.125 / (WSCALE * WSCALE)

    with tile.TileContext(nc) as tc:
        with (
            tc.tile_pool(name="res", bufs=1) as res,
            tc.tile_pool(name="acts", bufs=1) as acts,
            tc.tile_pool(name="work", bufs=2) as work,
            tc.tile_pool(name="outp", bufs=4) as outp,
            tc.tile_pool(name="psum", bufs=1, space="PSUM") as psum,
        ):
            # ---------------- resident input tiles ----------------
            qT = [res.tile([128, S], bf16, tag=f"qT{i}", name=f"qT{i}") for i in range(ND)]
            kT = [res.tile([128, S], bf16, tag=f"kT{i}", name=f"kT{i}") for i in range(ND)]
            vT = [res.tile([128, S], bf16, tag=f"vT{i}", name=f"vT{i}") for i in range(ND)]
            wqp = res.tile([128, ND * C], bf16, tag="wqp")
            wkp = res.tile([128, ND * C], bf16, tag="wkp")
            wvp = res.tile([128, ND * C], bf16, tag="wvp")
            wf = [res.tile([128, D], bf16, tag=f"wf{i}", name=f"wf{i}") for i in range(2)]
            wg4 = [res.tile([128, HPC], bf16, tag=f"wg4{i}", name=f"wg4{i}")
                   for i in range(2)]
            bq = res.tile([1, C], bf16, tag="bq")
            bk = res.tile([1, C], bf16, tag="bk")
            bv = res.tile([1, C], bf16, tag="bv")
            bg128 = res.tile([128, 1], f32, tag="bg128")
            ones = res.tile([1, 512], bf16, tag="ones")

            def wsl(wt, dc, cb):
                return wt[:, dc * C + cb * 128: dc * C + (cb + 1) * 128]

            # ---------------- DMA emission (arrival order = priority) ------
            nc.sync.dma_start(wkp[:], wk_d[:, :])
            nc.sync.dma_start(bk[:], bk_d[:])
            nc.sync.dma_start(bg128[:], bg_d[:])
            for i in range(2):
                nc.sync.dma_start(wg4[i][:], wg4_d[i * 128:(i + 1) * 128, :])
            for cols in (slice(0, 1024), slice(1024, 2048)):   # kT in halves
                for i in range(ND):
                    nc.sync.dma_start(kT[i][:, cols], kT_d[i * 128:(i + 1) * 128, cols])
            nc.sync.dma_start(wqp[:], wq_d[:, :])
            nc.sync.dma_start(bq[:], bq_d[:])
            for i in range(ND):   # qT first half (Q proj a=0)
                nc.sync.dma_start(qT[i][:, 0:1024], qT_d[i * 128:(i + 1) * 128, 0:1024])
            nc.sync.dma_start(wvp[:], wv_d[:, :])
            nc.sync.dma_start(bv[:], bv_d[:])
            for cols in (slice(0, 1024), slice(1024, 2048)):
                for i in range(ND):
                    nc.sync.dma_start(vT[i][:, cols], vT_d[i * 128:(i + 1) * 128, cols])
            for i in range(ND):   # qT second half
                nc.sync.dma_start(qT[i][:, 1024:2048],
                                  qT_d[i * 128:(i + 1) * 128, 1024:2048])
            for i in range(2):
                nc.sync.dma_start(wf[i][:], wf_d[i * 128:(i + 1) * 128, :])
            nc.vector.memset(ones[:], 1.0)

            # ---------------- persistent activation tiles ----------------
            QT = [acts.tile([128, S], bf16, tag=f"QT{i}", name=f"QT{i}") for i in range(2)]
            KT = [acts.tile([128, S], bf16, tag=f"KT{i}", name=f"KT{i}") for i in range(2)]
            ctxT = [acts.tile([128, S], bf16, tag=f"ctxT{i}", name=f"ctxT{i}")
                    for i in range(2)]
            gscT = acts.tile([128, NSB * HPC], f32, tag="gscT")
            V = [acts.tile([128, HPC * 2 * DH], bf16, tag=f"V{i}", name=f"V{i}")
                 for i in range(NSB)]

            st_alt = [0]

            def project_chunk(wt, x_tiles, bias, dest, cb, qs, pname,
                              tag=None, skip_gc=False):
                # one 512-wide output chunk dest[:, qs] (+ rank-1 bias)
                if tag is None:
                    tag = f"st{st_alt[0] % 2}"
                    st_alt[0] += 1
                ps = psum.tile([128, 512], f32, tag=tag, name=pname)
                for dc in range(ND):
                    nc.tensor.matmul(
                        ps[:], wsl(wt, dc, cb), x_tiles[dc][:, qs],
                        start=(dc == 0), stop=False, skip_group_check=skip_gc,
                    )
                nc.tensor.matmul(
                    ps[:], bias[0:1, cb * 128:(cb + 1) * 128], ones[0:1, 0:512],
                    start=False, stop=True, skip_group_check=skip_gc,
                )
                nc.vector.tensor_copy(dest[:, qs], ps[:])

            def emit_vproj(sb, tag):
                nc.vector.memset(V[sb][:], 1.0)
                ps = psum.tile([128, C], f32, tag=tag, name=f"psv{sb}")
                for dc in range(ND):
                    nc.tensor.matmul(
                        ps[:], vT[dc][:, sb * 128:(sb + 1) * 128],
                        wvp[:, dc * C:(dc + 1) * C],
                        start=(dc == 0), stop=False, skip_group_check=True,
                    )
                nc.tensor.matmul(ps[:], ones[0:1, 0:128], bv[:], start=False,
                                 stop=True, skip_group_check=True)
                nc.vector.tensor_copy(
                    V[sb][:].rearrange("p (h x) -> p h x", h=HPC)[:, :, 0:DH],
                    ps[:].rearrange("p (h x) -> p h x", h=HPC),
                )

            def emit_outproj(a, qb, tag, cast_engine=None):
                # out rows [a*1024 + qb*128 : +128]; contract ctx dims (2 cb)
                ps = psum.tile([128, 1024], f32, tag=tag, name=f"pso{a}{qb}")
                row0 = a * 1024 + qb * 128
                for oc in range(2):
                    for cc in range(2):
                        nc.tensor.matmul(
                            ps[:, oc * 512:(oc + 1) * 512],
                            ctxT[cc][:, row0:row0 + 128],
                            wf[cc][:, oc * 512:(oc + 1) * 512],
                            start=(cc == 0), stop=(cc == 1),
                            skip_group_check=True,
                        )
                ob = outp.tile([128, D], bf16, tag="ob", name=f"ob{a}{qb}")
                if cast_engine == "scalar":
                    nc.scalar.activation(ob[:], ps[:], Act.Copy)
                else:
                    nc.vector.tensor_copy(ob[:], ps[:])
                nc.sync.dma_start(out_d[row0:row0 + 128, :], ob[:])

            # ---------------- prephase: K proj + gates pipelined with DMA --
            gps = psum.tile([128, NSB * HPC], f32, tag="aux0", name="gps")
            gn = work.tile([128, NSB * HPC], f32, tag="gn", bufs=1)
            for t in range(4):
                ts_ = slice(t * 512, (t + 1) * 512)
                for cb in range(2):
                    project_chunk(wkp, kT, bk, KT[cb], cb, ts_, f"psk{cb}{t}")
                for sb in range(4 * t, 4 * t + 4):
                    for cc in range(2):
                        nc.tensor.matmul(
                            gps[:, sb * HPC:(sb + 1) * HPC],
                            KT[cc][:, sb * 128:(sb + 1) * 128],
                            wg4[cc][:],
                            start=(cc == 0), stop=(cc == 1),
                        )
                gsl = slice(4 * t * HPC, (4 * t + 4) * HPC)
                nc.scalar.activation(gn[:, gsl], gps[:, gsl], Act.Sigmoid,
                                     bias=bg128[:], scale=1.0)
                nc.vector.tensor_scalar(gscT[:, gsl], gn[:, gsl], GSC, GSC,
                                        Alu.mult, Alu.add)

            # Q proj for cb0, a=0 (first head's rhs)
            for t in range(2):
                project_chunk(wqp, qT, bq, QT[0], 0, slice(t * 512, (t + 1) * 512),
                              f"psq00{t}")

            # ---------------- pop schedule ----------------
            pops = {}

            def add_pop(si, kb, fn):
                pops.setdefault((si, kb), []).append(fn)

            # V proj during head 0 (its PV(kb>=8) are deferred to head 1)
            for sb in range(NSB - 1):
                add_pop(0, sb + 1, (lambda sb=sb: emit_vproj(sb, "aux1")))
            def qproj_job(cb, ah, tag):
                # both 512-chunks of QT[cb] half `ah` in one psum tile with a
                # single PSUM-freeing cast (two casts would serialize the
                # next job's matmuls behind the DVE queue)
                ps = psum.tile([128, 1024], f32, tag=tag, name=f"psq{cb}{ah}")
                for t in range(2):
                    ph = ps[:, t * 512:(t + 1) * 512]
                    qs = slice(ah * 1024 + t * 512, ah * 1024 + (t + 1) * 512)
                    for dc in range(ND):
                        nc.tensor.matmul(
                            ph, wsl(wqp, dc, cb), qT[dc][:, qs],
                            start=(dc == 0), stop=False, skip_group_check=True)
                    nc.tensor.matmul(
                        ph, bq[0:1, cb * 128:(cb + 1) * 128], ones[0:1, 0:512],
                        start=False, stop=True, skip_group_check=True)
                nc.vector.tensor_copy(
                    QT[cb][:, ah * 1024:(ah + 1) * 1024], ps[:])

            # Q proj cb1 a0 (needed at si=2): aux0 freed by otc(0,0) @ (1,kb9)
            add_pop(1, 11, lambda: qproj_job(1, 0, "aux0"))
            # Q proj a1 (needed at si=4/si=6): aux1 freed by otc(0,1) @ (2,kb1)
            add_pop(2, 3, lambda: qproj_job(0, 1, "aux1"))
            add_pop(2, 8, lambda: qproj_job(1, 1, "aux1"))
            # out-proj of half 0: after normfin(0,3) completes ~ (4,kb8)
            for j in range(4):
                add_pop(4, 9 + 2 * j, (lambda j=j: emit_outproj(0, j, "aux1")))
            for j in range(4, 8):
                add_pop(5, 2 + 3 * (j - 4),
                        (lambda j=j: emit_outproj(0, j, "aux0")))

            # ---------------- attention: continuous cross-head pipeline ----
            seq = [(a, h) for a in range(NA) for h in range(HPC)]
            # normfin(si) pop slots: (target_si, [kb_c0, kb_c1])
            NF_SLOT = {0: (2, (2, 4)), 1: (3, (2, 4)), 2: (3, (6, 8)),
                       3: (4, (2, 4)), 4: (6, (2, 4)), 5: (7, (2, 4)),
                       6: (7, (6, 8))}
            carry = []   # closures from the previous head, one per early iter

            for si, (a, h) in enumerate(seq):
                qs = slice(a * AW, (a + 1) * AW)
                cb, po = h // 2, (h % 2) * 64
                vsl = slice(h * 2 * DH, (h + 1) * 2 * DH)
                ot = psum.tile([128, AW], f32, tag=f"aux{h % 2}",
                               name=f"ot{a}{h}")
                defer = 8 if si == 0 else 15   # PV(kb>=defer) carried onward
                pts = {}

                def mk_pv(j, ot=ot, vsl=vsl, pts=pts, defer=defer):
                    def f():
                        ptp = pts.pop(j)
                        for qc in range(2):
                            cs = slice(qc * 512, (qc + 1) * 512)
                            nc.tensor.matmul(
                                ot[:, cs], V[j][:, vsl], ptp[:, cs],
                                start=(j == 0), stop=(j == NSB - 1),
                                skip_group_check=True)
                    return f

                for kb in range(NSB):
                    kslc = slice(kb * 128, (kb + 1) * 128)
                    st = psum.tile([128, AW], f32, tag=f"st{kb % 2}",
                                   name=f"st{a}{h}{kb}")
                    for qc in range(2):
                        cs = slice(qc * 512, (qc + 1) * 512)
                        aqs = slice(a * AW + qc * 512, a * AW + (qc + 1) * 512)
                        nc.tensor.matmul(st[:, cs], KT[cb][po:po + 64, kslc],
                                         QT[cb][po:po + 64, aqs],
                                         start=True, stop=True,
                                         skip_group_check=True)
                    pt = work.tile([128, AW], bf16, tag="pt",
                                   name=f"pt{a}{h}{kb}", bufs=10)
                    nc.scalar.activation(pt[:], st[:], Act.Exp,
                                         scale=gscT[:, kb * HPC + h:kb * HPC + h + 1])
                    pts[kb] = pt
                    if carry:
                        carry.pop(0)()
                    for fn in pops.get((si, kb), ()):
                        fn()
                    if 1 <= kb and kb - 1 < defer:
                        mk_pv(kb - 1)()

                # hand the rest of this head to the next heads' iterations
                newcarry = [mk_pv(j) for j in range(defer, NSB)]
                if si == 0:
                    newcarry.insert(0, lambda: emit_vproj(NSB - 1, "aux1"))

                def mk_otc(si=si, a=a, h=h, cb=cb, po=po, ot=ot):
                    otc = work.tile([128, AW], f32, tag="otc",
                                    name=f"otc{a}{h}", bufs=2)
                    nc.vector.tensor_copy(otc[:], ot[:])

                    def norm_fin_chunk(part):
                        rec = work.tile([DH, 512], f32, tag=f"rec{part}",
                                        name=f"rec{a}{h}{part}", bufs=2)
                        cs = slice(part * 512, (part + 1) * 512)
                        qcs = slice(a * AW + part * 512, a * AW + (part + 1) * 512)
                        with tc.high_priority(offset=-256):
                            nc.vector.reciprocal(rec[:], otc[DH:2 * DH, cs])
                            nc.vector.tensor_tensor(
                                ctxT[cb][po:po + 64, qcs], otc[0:DH, cs],
                                rec[:], Alu.mult)

                    if si in NF_SLOT:
                        tsi, kbs = NF_SLOT[si]
                        add_pop(tsi, kbs[0], lambda: norm_fin_chunk(0))
                        add_pop(tsi, kbs[1], lambda: norm_fin_chunk(1))
                        return None
                    return otc

                if si + 1 < len(seq):
                    newcarry.append(mk_otc)
                    carry = newcarry
                else:
                    # tail: drain the last PV, then normalize on the idle ACT
                    # engine: exp(-ln(den)), chunked mult + out-proj of half 1.
                    for c in newcarry:
                        c()
                    otcf = mk_otc()
                    lt = work.tile([DH, AW], f32, tag="ltf", name="lt", bufs=1)
                    rec = work.tile([DH, AW], f32, tag="recf", name="recf", bufs=1)
                    nc.scalar.activation(lt[:], otcf[DH:2 * DH, :], Act.Ln)
                    nc.scalar.activation(rec[:], lt[:], Act.Exp, scale=-1.0)
                    for cchunk in range(4):
                        cs = slice(cchunk * 256, (cchunk + 1) * 256)
                        qcs = slice(a * AW + cchunk * 256,
                                    a * AW + (cchunk + 1) * 256)
                        nc.vector.tensor_tensor(
                            ctxT[cb][po:po + 64, qcs], otcf[0:DH, cs],
                            rec[:, cs], Alu.mult)
                        for qb in (2 * cchunk, 2 * cchunk + 1):
                            emit_outproj(1, qb, f"aux{qb % 2}",
                                         cast_engine=("scalar", None)[qb % 2])

    nc.finalize()
    return nc


def get_nc():
    if "nc" not in _nc_cache:
        _nc_cache["nc"] = build_bass()
    return _nc_cache["nc"]


def pack_w(W, cols):
    """[D, C] weight slice -> packed [128, ND*C] (dc-major along free)."""
    Wc = np.ascontiguousarray(np.asarray(W, np.float32)[:, cols])
    return np.concatenate([Wc[dc * 128:(dc + 1) * 128, :] for dc in range(ND)],
                          axis=1).astype(BF16)


def make_in_maps(query, key_, value, Wq, bq, Wk, bk, Wv, bv, wg, bg, Wo, bo, Wd, bd, Wh, bh):
    """Host-side sharding: returns (in_maps for 8 cores, fused bias)."""
    f = np.asarray
    Wf = f(Wo, np.float64) @ f(Wd, np.float64) @ f(Wh, np.float64)
    bf = (f(bo, np.float64) @ f(Wd, np.float64) @ f(Wh, np.float64)
          + f(bd, np.float64) @ f(Wh, np.float64) + f(bh, np.float64))

    wg4 = np.zeros((C, HPC), np.float32)
    for h in range(HPC):
        wg4[h * DH:(h + 1) * DH, h] = np.asarray(wg, np.float32)
    wg4 = wg4.astype(BF16)
    bg128 = np.full((128, 1), np.float32(bg), np.float32)

    xT = []
    for b in range(B):
        xT.append(tuple(
            np.ascontiguousarray(np.asarray(x[b], np.float32).T).astype(BF16)
            for x in (query, key_, value)
        ))

    in_maps = []
    for c in range(NCORES):
        b, g = divmod(c, HPC)
        cols = slice(g * C, (g + 1) * C)
        qTb, kTb, vTb = xT[b]
        in_maps.append({
            "qT": qTb, "kT": kTb, "vT": vTb,
            "wq": pack_w(Wq, cols), "wk": pack_w(Wk, cols), "wv": pack_w(Wv, cols),
            "wf": np.ascontiguousarray(Wf[cols, :]).astype(BF16),
            "wg4": wg4, "bg128": bg128,
            "bq": np.asarray(bq, np.float32)[None, cols].astype(BF16),
            "bk": np.asarray(bk, np.float32)[None, cols].astype(BF16),
            "bv": np.asarray(bv, np.float32)[None, cols].astype(BF16),
        })
    return in_maps, bf.astype(np.float32)


def gather(results, bf):
    out = np.zeros((B, S, D), np.float32)
    for c in range(NCORES):
        b = c // HPC
        out[b] += np.asarray(results[c]["out"], np.float32)
    out += bf[None, None, :]
    return out


def kernel(**inputs):
    from concourse.bass_utils import run_bass_kernel_spmd

    nc = get_nc()
    in_maps, bf = make_in_maps(**inputs)
    res = run_bass_kernel_spmd(nc, in_maps, core_ids=list(range(NCORES)))
    return gather(res.results, bf)


# revision 18
# speedup vs baseline: 1.1833x; 1.0253x over previous
"""Trainium2 Bass kernel for MultiHeadSyntonicAttention.

Problem: B=2, S=2048, D=1024, H=16 heads, DH=64.
  q/k/v = Linear(query/key/value); per-head gnosis gate
  gn = sigmoid(k . wg + bg); scores = (q k^T / sqrt(dh)) * (1+gn);
  out = softmax(scores) v;  out = ((out Wo+bo) Wd+bd) Wh+bh.

Sharding (8 cores): core c -> batch b=c//4, head-group g=c%4 (4 heads).
Each core computes its heads' attention and a row-slice partial of the
fused output projection Wf = Wo@Wd@Wh; host sums 4 partials per batch.

Device layout (everything "transposed", tokens on the free axis):
  QT[cb][128, S] bf16 (2 heads' dh on partitions) = Wq_s^T qT
  ST[k=128, q=1024] per (a-half, head, kb) f32 in PSUM
  pt = exp(ST * gsc) bf16  (per-partition scale; max-sub skipped)
  ot[128, 1024] accum over kb: lhsT=[V_h | ones] -> rows 64:128 = denom
  ctxT = otc[0:64] * recip(otc[64:128]) -> out = ctxT^T Wf (+host bf)

Schedule: one continuous ACT(exp)-paced software pipeline across all 8
(a-half, head) passes.  Each head's last PV / PSUM-drain copy / the
reciprocal+mult normalize-finish are carried into the NEXT heads' kb
iterations as scheduled pops, so no PE matmul ever queues behind a
DVE-FIFO bubble and the next head's QK/exp start immediately at each
boundary.  Head 0 additionally defers PV(kb>=8) into head 1 (head 0's
iterations carry the 16 V-projections).  Half-0's output projection +
DMA-out run during half-1's attention; the final normalize runs on the
then-idle ACT engine as exp(-ln(den)) feeding a chunked tail.
PSUM: st0/st1 double-buffered score tiles + aux0/aux1 alternating PV
accumulators; background psum borrows the idle aux tag.  Weights ship
packed [128, 2048] (4KB DMA rows; 512B-row transfers run ~4x slower).
"""

import sys

sys.path.insert(0, "/opt/trn_rl_repo")

import numpy as np
import ml_dtypes

BF16 = ml_dtypes.bfloat16
WSCALE = 1.0   # fp8 shipping fails the 2e-2 bar (softmax ctx is sqrt(N)-
               # suppressed; elementwise noise passes through at full size)

B, S, D, H = 2, 2048, 1024, 16
DH = D // H          # 64
HPC = 4              # heads per core
C = HPC * DH         # 256 head-local columns per core
NCORES = 8
ND = D // 128        # 8 d-chunks
NSB = S // 128       # 16 s-blocks
AW = 1024            # attention q-tile width (q-half)
NA = S // AW         # 2

_nc_cache = {}


def build_bass():
    import concourse.bass as bass
    import concourse.mybir as mybir
    import concourse.tile as tile
    from concourse import bacc

    f32 = mybir.dt.float32
    bf16 = mybir.dt.bfloat16
    Alu = mybir.AluOpType
    Act = mybir.ActivationFunctionType

    nc = bacc.Bacc(None, target_bir_lowering=False, name="syntonic_attn")

    qT_d = nc.dram_tensor("qT", [D, S], bf16, kind="ExternalInput")
    kT_d = nc.dram_tensor("kT", [D, S], bf16, kind="ExternalInput")
    vT_d = nc.dram_tensor("vT", [D, S], bf16, kind="ExternalInput")
    wq_d = nc.dram_tensor("wq", [128, ND * C], bf16, kind="ExternalInput")
    wk_d = nc.dram_tensor("wk", [128, ND * C], bf16, kind="ExternalInput")
    wv_d = nc.dram_tensor("wv", [128, ND * C], bf16, kind="ExternalInput")
    wf_d = nc.dram_tensor("wf", [C, D], bf16, kind="ExternalInput")
    wg4_d = nc.dram_tensor("wg4", [C, HPC], bf16, kind="ExternalInput")
    bq_d = nc.dram_tensor("bq", [1, C], bf16, kind="ExternalInput")
    bk_d = nc.dram_tensor("bk", [1, C], bf16, kind="ExternalInput")
    bv_d = nc.dram_tensor("bv", [1, C], bf16, kind="ExternalInput")
    bg_d = nc.dram_tensor("bg128", [128, 1], f32, kind="ExternalInput")
    out_d = nc.dram_tensor("out", [S, D], bf16, kind="ExternalOutput")

    GSC = 0.125 / (WSCALE * WSCALE)

    with tile.TileContext(nc) as tc:
        with (
            tc.tile_pool(name="res", bufs=1) as res,
            tc.tile_pool(name="acts", bufs=1) as acts,
            tc.tile_pool(name="work", bufs=2) as work,
            tc.tile_pool(name="outp", bufs=4) as outp,
            tc.tile_pool(name="psum", bufs=1, space="PSUM") as psum,
        ):
            # ---------------- resident input tiles ----------------
            qT = [res.tile([128, S], bf16, tag=f"qT{i}", name=f"qT{i}") for i in range(ND)]
            kT = [res.tile([128, S], bf16, tag=f"kT{i}", name=f"kT{i}") for i in range(ND)]
            vT = [res.tile([128, S], bf16, tag=f"vT{i}", name=f"vT{i}") for i in range(ND)]
            wqp = res.tile([128, ND * C], bf16, tag="wqp")
            wkp = res.tile([128, ND * C], bf16, tag="wkp")
            wvp = res.tile([128, ND * C], bf16, tag="wvp")
            wf = [res.tile([128, D], bf16, tag=f"wf{i}", name=f"wf{i}") for i in range(2)]
            wg4 = [res.tile([128, HPC], bf16, tag=f"wg4{i}", name=f"wg4{i}")
                   for i in range(2)]
            bq = res.tile([1, C], bf16, tag="bq")
            bk = res.tile([1, C], bf16, tag="bk")
            bv = res.tile([1, C], bf16, tag="bv")
            bg128 = res.tile([128, 1], f32, tag="bg128")
            ones = res.tile([1, 512], bf16, tag="ones")

            def wsl(wt, dc, cb):
                return wt[:, dc * C + cb * 128: dc * C + (cb + 1) * 128]

            # ---------------- DMA emission (arrival order = priority) ------
            nc.sync.dma_start(wkp[:], wk_d[:, :])
            nc.sync.dma_start(bk[:], bk_d[:])
            nc.sync.dma_start(bg128[:], bg_d[:])
            for i in range(2):
                nc.sync.dma_start(wg4[i][:], wg4_d[i * 128:(i + 1) * 128, :])
            for cols in (slice(0, 1024), slice(1024, 2048)):   # kT in halves
                for i in range(ND):
                    nc.sync.dma_start(kT[i][:, cols], kT_d[i * 128:(i + 1) * 128, cols])
            nc.sync.dma_start(wqp[:], wq_d[:, :])
            nc.sync.dma_start(bq[:], bq_d[:])
            nc.sync.dma_start(wvp[:], wv_d[:, :])
            nc.sync.dma_start(bv[:], bv_d[:])
            for i in range(ND):   # vT first quarter: V-proj 0..3 fill the
                nc.sync.dma_start(vT[i][:, 0:512],   # prephase PE-idle window
                                  vT_d[i * 128:(i + 1) * 128, 0:512])
            for i in range(ND):   # qT first half (Q proj a=0)
                nc.sync.dma_start(qT[i][:, 0:1024], qT_d[i * 128:(i + 1) * 128, 0:1024])
            for cols in (slice(512, 1024), slice(1024, 2048)):
                for i in range(ND):
                    nc.sync.dma_start(vT[i][:, cols], vT_d[i * 128:(i + 1) * 128, cols])
            for i in range(ND):   # qT second half
                nc.sync.dma_start(qT[i][:, 1024:2048],
                                  qT_d[i * 128:(i + 1) * 128, 1024:2048])
            for i in range(2):
                nc.sync.dma_start(wf[i][:], wf_d[i * 128:(i + 1) * 128, :])
            nc.vector.memset(ones[:], 1.0)

            # ---------------- persistent activation tiles ----------------
            QT = [acts.tile([128, S], bf16, tag=f"QT{i}", name=f"QT{i}") for i in range(2)]
            KT = [acts.tile([128, S], bf16, tag=f"KT{i}", name=f"KT{i}") for i in range(2)]
            ctxT = [acts.tile([128, S], bf16, tag=f"ctxT{i}", name=f"ctxT{i}")
                    for i in range(2)]
            gscT = acts.tile([128, NSB * HPC], f32, tag="gscT")
            V = [acts.tile([128, HPC * 2 * DH], bf16, tag=f"V{i}", name=f"V{i}")
                 for i in range(NSB)]

            st_alt = [0]

            def project_chunk(wt, x_tiles, bias, dest, cb, qs, pname,
                              tag=None, skip_gc=False):
                # one 512-wide output chunk dest[:, qs] (+ rank-1 bias)
                if tag is None:
                    tag = f"st{st_alt[0] % 2}"
                    st_alt[0] += 1
                ps = psum.tile([128, 512], f32, tag=tag, name=pname)
                for dc in range(ND):
                    nc.tensor.matmul(
                        ps[:], wsl(wt, dc, cb), x_tiles[dc][:, qs],
                        start=(dc == 0), stop=False, skip_group_check=skip_gc,
                    )
                nc.tensor.matmul(
                    ps[:], bias[0:1, cb * 128:(cb + 1) * 128], ones[0:1, 0:512],
                    start=False, stop=True, skip_group_check=skip_gc,
                )
                nc.vector.tensor_copy(dest[:, qs], ps[:])

            def emit_vproj(sb, tag):
                nc.vector.memset(V[sb][:], 1.0)
                ps = psum.tile([128, C], f32, tag=tag, name=f"psv{sb}")
                for dc in range(ND):
                    nc.tensor.matmul(
                        ps[:], vT[dc][:, sb * 128:(sb + 1) * 128],
                        wvp[:, dc * C:(dc + 1) * C],
                        start=(dc == 0), stop=False, skip_group_check=True,
                    )
                nc.tensor.matmul(ps[:], ones[0:1, 0:128], bv[:], start=False,
                                 stop=True, skip_group_check=True)
                nc.vector.tensor_copy(
                    V[sb][:].rearrange("p (h x) -> p h x", h=HPC)[:, :, 0:DH],
                    ps[:].rearrange("p (h x) -> p h x", h=HPC),
                )

            def emit_outproj(a, qb, tag, cast_engine=None):
                # out rows [a*1024 + qb*128 : +128]; contract ctx dims (2 cb)
                ps = psum.tile([128, 1024], f32, tag=tag, name=f"pso{a}{qb}")
                row0 = a * 1024 + qb * 128
                for oc in range(2):
                    for cc in range(2):
                        nc.tensor.matmul(
                            ps[:, oc * 512:(oc + 1) * 512],
                            ctxT[cc][:, row0:row0 + 128],
                            wf[cc][:, oc * 512:(oc + 1) * 512],
                            start=(cc == 0), stop=(cc == 1),
                            skip_group_check=True,
                        )
                ob = outp.tile([128, D], bf16, tag="ob", name=f"ob{a}{qb}")
                if cast_engine == "scalar":
                    nc.scalar.activation(ob[:], ps[:], Act.Copy)
                else:
                    nc.vector.tensor_copy(ob[:], ps[:])
                nc.sync.dma_start(out_d[row0:row0 + 128, :], ob[:])

            # ---------------- prephase: K proj + gates pipelined with DMA --
            gps = psum.tile([128, NSB * HPC], f32, tag="aux0", name="gps")
            gn = work.tile([128, NSB * HPC], f32, tag="gn", bufs=1)
            for t in range(4):
                ts_ = slice(t * 512, (t + 1) * 512)
                for cb in range(2):
                    project_chunk(wkp, kT, bk, KT[cb], cb, ts_, f"psk{cb}{t}")
                for sb in range(4 * t, 4 * t + 4):
                    for cc in range(2):
                        nc.tensor.matmul(
                            gps[:, sb * HPC:(sb + 1) * HPC],
                            KT[cc][:, sb * 128:(sb + 1) * 128],
                            wg4[cc][:],
                            start=(cc == 0), stop=(cc == 1),
                        )
                gsl = slice(4 * t * HPC, (4 * t + 4) * HPC)
                nc.scalar.activation(gn[:, gsl], gps[:, gsl], Act.Sigmoid,
                                     bias=bg128[:], scale=1.0)
                nc.vector.tensor_scalar(gscT[:, gsl], gn[:, gsl], GSC, GSC,
                                        Alu.mult, Alu.add)

            # V proj 0..3 in the DMA-wait window, then Q proj cb0-a0
            for sb in range(4):
                emit_vproj(sb, "aux1")
            for t in range(2):
                project_chunk(wqp, qT, bq, QT[0], 0, slice(t * 512, (t + 1) * 512),
                              f"psq00{t}")

            # ---------------- pop schedule ----------------
            pops = {}

            def add_pop(si, kb, fn):
                pops.setdefault((si, kb), []).append(fn)

            # V proj 4..14 during head 0 (its PV(kb>=8) defer to head 1)
            for sb in range(4, NSB - 1):
                add_pop(0, sb - 3, (lambda sb=sb: emit_vproj(sb, "aux1")))
            def qproj_job(cb, ah, tag):
                # both 512-chunks of QT[cb] half `ah` in one psum tile with a
                # single PSUM-freeing cast (two casts would serialize the
                # next job's matmuls behind the DVE queue)
                ps = psum.tile([128, 1024], f32, tag=tag, name=f"psq{cb}{ah}")
                for t in range(2):
                    ph = ps[:, t * 512:(t + 1) * 512]
                    qs = slice(ah * 1024 + t * 512, ah * 1024 + (t + 1) * 512)
                    for dc in range(ND):
                        nc.tensor.matmul(
                            ph, wsl(wqp, dc, cb), qT[dc][:, qs],
                            start=(dc == 0), stop=False, skip_group_check=True)
                    nc.tensor.matmul(
                        ph, bq[0:1, cb * 128:(cb + 1) * 128], ones[0:1, 0:512],
                        start=False, stop=True, skip_group_check=True)
                nc.vector.tensor_copy(
                    QT[cb][:, ah * 1024:(ah + 1) * 1024], ps[:])

            # Q proj cb1 a0 (needed at si=2): aux0 freed by otc(0,0) @ (1,kb9)
            add_pop(1, 11, lambda: qproj_job(1, 0, "aux0"))
            # Q proj a1 (needed at si=4/si=6): aux1 freed by otc(0,1) @ (2,kb1)
            add_pop(2, 3, lambda: qproj_job(0, 1, "aux1"))
            add_pop(2, 8, lambda: qproj_job(1, 1, "aux1"))
            # out-proj of half 0: after normfin(0,3) completes ~ (4,kb8)
            for j in range(4):
                add_pop(4, 9 + 2 * j, (lambda j=j: emit_outproj(0, j, "aux1")))
            for j in range(4, 8):
                add_pop(5, 2 + 3 * (j - 4),
                        (lambda j=j: emit_outproj(0, j, "aux0")))

            # ---------------- attention: continuous cross-head pipeline ----
            seq = [(a, h) for a in range(NA) for h in range(HPC)]
            # normfin(si) pop slots: (target_si, [kb_c0, kb_c1])
            NF_SLOT = {0: (2, (2, 4)), 1: (3, (2, 4)), 2: (3, (6, 8)),
                       3: (4, (2, 4)), 4: (6, (2, 4)), 5: (7, (2, 4)),
                       6: (7, (6, 8))}
            carry = []   # closures from the previous head, one per early iter

            for si, (a, h) in enumerate(seq):
                qs = slice(a * AW, (a + 1) * AW)
                cb, po = h // 2, (h % 2) * 64
                vsl = slice(h * 2 * DH, (h + 1) * 2 * DH)
                ot = psum.tile([128, AW], f32, tag=f"aux{h % 2}",
                               name=f"ot{a}{h}")
                defer = 8 if si == 0 else 15   # PV(kb>=defer) carried onward
                pts = {}

                def mk_pv(j, ot=ot, vsl=vsl, pts=pts, defer=defer):
                    def f():
                        ptp = pts.pop(j)
                        for qc in range(2):
                            cs = slice(qc * 512, (qc + 1) * 512)
                            nc.tensor.matmul(
                                ot[:, cs], V[j][:, vsl], ptp[:, cs],
                                start=(j == 0), stop=(j == NSB - 1),
                                skip_group_check=True)
                    return f

                for kb in range(NSB):
                    kslc = slice(kb * 128, (kb + 1) * 128)
                    st = psum.tile([128, AW], f32, tag=f"st{kb % 2}",
                                   name=f"st{a}{h}{kb}")
                    for qc in range(2):
                        cs = slice(qc * 512, (qc + 1) * 512)
                        aqs = slice(a * AW + qc * 512, a * AW + (qc + 1) * 512)
                        nc.tensor.matmul(st[:, cs], KT[cb][po:po + 64, kslc],
                                         QT[cb][po:po + 64, aqs],
                                         start=True, stop=True,
                                         skip_group_check=True)
                    pt = work.tile([128, AW], bf16, tag="pt",
                                   name=f"pt{a}{h}{kb}", bufs=10)
                    nc.scalar.activation(pt[:], st[:], Act.Exp,
                                         scale=gscT[:, kb * HPC + h:kb * HPC + h + 1])
                    pts[kb] = pt
                    if carry:
                        carry.pop(0)()
                    for fn in pops.get((si, kb), ()):
                        fn()
                    if 1 <= kb and kb - 1 < defer:
                        mk_pv(kb - 1)()

                # hand the rest of this head to the next heads' iterations
                newcarry = [mk_pv(j) for j in range(defer, NSB)]
                if si == 0:
                    newcarry.insert(0, lambda: emit_vproj(NSB - 1, "aux1"))

                def mk_otc(si=si, a=a, h=h, cb=cb, po=po, ot=ot):
                    otc = work.tile([128, AW], f32, tag="otc",
                                    name=f"otc{a}{h}", bufs=2)
                    nc.vector.tensor_copy(otc[:], ot[:])

                    def norm_fin_chunk(part):
                        rec = work.tile([DH, 512], f32, tag=f"rec{part}",
                                        name=f"rec{a}{h}{part}", bufs=2)
                        cs = slice(part * 512, (part + 1) * 512)
                        qcs = slice(a * AW + part * 512, a * AW + (part + 1) * 512)
                        with tc.high_priority(offset=-256):
                            nc.vector.reciprocal(rec[:], otc[DH:2 * DH, cs])
                            nc.vector.tensor_tensor(
                                ctxT[cb][po:po + 64, qcs], otc[0:DH, cs],
                                rec[:], Alu.mult)

                    if si in NF_SLOT:
                        tsi, kbs = NF_SLOT[si]
                        add_pop(tsi, kbs[0], lambda: norm_fin_chunk(0))
                        add_pop(tsi, kbs[1], lambda: norm_fin_chunk(1))
                        return None
                    return otc

                if si + 1 < len(seq):
                    newcarry.append(mk_otc)
                    carry = newcarry
                else:
                    # tail: drain the last PV, then normalize on the idle ACT
                    # engine: exp(-ln(den)), chunked mult + out-proj of half 1.
                    for c in newcarry:
                        c()
                    otcf = mk_otc()
                    lt = work.tile([DH, AW], f32, tag="ltf", name="lt", bufs=1)
                    rec = work.tile([DH, AW], f32, tag="recf", name="recf", bufs=1)
                    nc.scalar.activation(lt[:], otcf[DH:2 * DH, :], Act.Ln)
                    nc.scalar.activation(rec[:], lt[:], Act.Exp, scale=-1.0)
                    for cchunk in range(4):
                        cs = slice(cchunk * 256, (cchunk + 1) * 256)
                        qcs = slice(a * AW + cchunk * 256,
                                    a * AW + (cchunk + 1) * 256)
                        nc.vector.tensor_tensor(
                            ctxT[cb][po:po + 64, qcs], otcf[0:DH, cs],
                            rec[:, cs], Alu.mult)
                        for qb in (2 * cchunk, 2 * cchunk + 1):
                            emit_outproj(1, qb, f"aux{qb % 2}",
                                         cast_engine=("scalar", None)[qb % 2])

    nc.finalize()
    return nc


def get_nc():
    if "nc" not in _nc_cache:
        _nc_cache["nc"] = build_bass()
    return _nc_cache["nc"]


def pack_w(W, cols):
    """[D, C] weight slice -> packed [128, ND*C] (dc-major along free)."""
    Wc = np.ascontiguousarray(np.asarray(W, np.float32)[:, cols])
    return np.concatenate([Wc[dc * 128:(dc + 1) * 128, :] for dc in range(ND)],
                          axis=1).astype(BF16)


def make_in_maps(query, key_, value, Wq, bq, Wk, bk, Wv, bv, wg, bg, Wo, bo, Wd, bd, Wh, bh):
    """Host-side sharding: returns (in_maps for 8 cores, fused bias)."""
    f = np.asarray
    Wf = f(Wo, np.float64) @ f(Wd, np.float64) @ f(Wh, np.float64)
    bf = (f(bo, np.float64) @ f(Wd, np.float64) @ f(Wh, np.float64)
          + f(bd, np.float64) @ f(Wh, np.float64) + f(bh, np.float64))

    wg4 = np.zeros((C, HPC), np.float32)
    for h in range(HPC):
        wg4[h * DH:(h + 1) * DH, h] = np.asarray(wg, np.float32)
    wg4 = wg4.astype(BF16)
    bg128 = np.full((128, 1), np.float32(bg), np.float32)

    xT = []
    for b in range(B):
        xT.append(tuple(
            np.ascontiguousarray(np.asarray(x[b], np.float32).T).astype(BF16)
            for x in (query, key_, value)
        ))

    in_maps = []
    for c in range(NCORES):
        b, g = divmod(c, HPC)
        cols = slice(g * C, (g + 1) * C)
        qTb, kTb, vTb = xT[b]
        in_maps.append({
            "qT": qTb, "kT": kTb, "vT": vTb,
            "wq": pack_w(Wq, cols), "wk": pack_w(Wk, cols), "wv": pack_w(Wv, cols),
            "wf": np.ascontiguousarray(Wf[cols, :]).astype(BF16),
            "wg4": wg4, "bg128": bg128,
            "bq": np.asarray(bq, np.float32)[None, cols].astype(BF16),
            "bk": np.asarray(bk, np.float32)[None, cols].astype(BF16),
            "bv": np.asarray(bv, np.float32)[None, cols].astype(BF16),
        })
    return in_maps, bf.astype(np.float32)


def gather(results, bf):
    out = np.zeros((B, S, D), np.float32)
    for c in range(NCORES):
        b = c // HPC
        out[b] += np.asarray(results[c]["out"], np.float32)
    out += bf[None, None, :]
    return out


def kernel(**inputs):
    from concourse.bass_utils import run_bass_kernel_spmd

    nc = get_nc()
    in_maps, bf = make_in_maps(**inputs)
    res = run_bass_kernel_spmd(nc, in_maps, core_ids=list(range(NCORES)))
    return gather(res.results, bf)


# revision 19
# speedup vs baseline: 1.1927x; 1.0080x over previous
"""Trainium2 Bass kernel for MultiHeadSyntonicAttention.

Problem: B=2, S=2048, D=1024, H=16 heads, DH=64.
  q/k/v = Linear(query/key/value); per-head gnosis gate
  gn = sigmoid(k . wg + bg); scores = (q k^T / sqrt(dh)) * (1+gn);
  out = softmax(scores) v;  out = ((out Wo+bo) Wd+bd) Wh+bh.

Sharding (8 cores): core c -> batch b=c//4, head-group g=c%4 (4 heads).
Each core computes its heads' attention and a row-slice partial of the
fused output projection Wf = Wo@Wd@Wh; host sums 4 partials per batch.

Device layout (everything "transposed", tokens on the free axis):
  QT[cb][128, S] bf16 (2 heads' dh on partitions) = Wq_s^T qT
  ST[k=128, q=1024] per (a-half, head, kb) f32 in PSUM
  pt = exp(ST * gsc) bf16  (per-partition scale; max-sub skipped)
  ot[128, 1024] accum over kb: lhsT=[V_h | ones] -> rows 64:128 = denom
  ctxT = otc[0:64] * recip(otc[64:128]) -> out = ctxT^T Wf (+host bf)

Schedule: one continuous ACT(exp)-paced software pipeline across all 8
(a-half, head) passes.  Each head's last PV / PSUM-drain copy / the
reciprocal+mult normalize-finish are carried into the NEXT heads' kb
iterations as scheduled pops, so no PE matmul ever queues behind a
DVE-FIFO bubble and the next head's QK/exp start immediately at each
boundary.  Head 0 additionally defers PV(kb>=8) into head 1 (head 0's
iterations carry the 16 V-projections).  Half-0's output projection +
DMA-out run during half-1's attention; the final normalize runs on the
then-idle ACT engine as exp(-ln(den)) feeding a chunked tail.
PSUM: st0/st1 double-buffered score tiles + aux0/aux1 alternating PV
accumulators; background psum borrows the idle aux tag.  Weights ship
packed [128, 2048] (4KB DMA rows; 512B-row transfers run ~4x slower).
"""

import sys

sys.path.insert(0, "/opt/trn_rl_repo")

import numpy as np
import ml_dtypes

BF16 = ml_dtypes.bfloat16
WSCALE = 1.0   # fp8 shipping fails the 2e-2 bar (softmax ctx is sqrt(N)-
               # suppressed; elementwise noise passes through at full size)

B, S, D, H = 2, 2048, 1024, 16
DH = D // H          # 64
HPC = 4              # heads per core
C = HPC * DH         # 256 head-local columns per core
NCORES = 8
ND = D // 128        # 8 d-chunks
NSB = S // 128       # 16 s-blocks
AW = 1024            # attention q-tile width (q-half)
NA = S // AW         # 2

_nc_cache = {}


def build_bass():
    import concourse.bass as bass
    import concourse.mybir as mybir
    import concourse.tile as tile
    from concourse import bacc

    f32 = mybir.dt.float32
    bf16 = mybir.dt.bfloat16
    Alu = mybir.AluOpType
    Act = mybir.ActivationFunctionType

    nc = bacc.Bacc(None, target_bir_lowering=False, name="syntonic_attn")

    qT_d = nc.dram_tensor("qT", [D, S], bf16, kind="ExternalInput")
    kT_d = nc.dram_tensor("kT", [D, S], bf16, kind="ExternalInput")
    vT_d = nc.dram_tensor("vT", [D, S], bf16, kind="ExternalInput")
    wq_d = nc.dram_tensor("wq", [128, ND * C], bf16, kind="ExternalInput")
    wk_d = nc.dram_tensor("wk", [128, ND * C], bf16, kind="ExternalInput")
    wv_d = nc.dram_tensor("wv", [128, ND * C], bf16, kind="ExternalInput")
    wf_d = nc.dram_tensor("wf", [C, D], bf16, kind="ExternalInput")
    wg4_d = nc.dram_tensor("wg4", [C, HPC], bf16, kind="ExternalInput")
    bq_d = nc.dram_tensor("bq", [1, C], bf16, kind="ExternalInput")
    bk_d = nc.dram_tensor("bk", [1, C], bf16, kind="ExternalInput")
    bv_d = nc.dram_tensor("bv", [1, C], bf16, kind="ExternalInput")
    bg_d = nc.dram_tensor("bg128", [128, 1], f32, kind="ExternalInput")
    out_d = nc.dram_tensor("out", [S, D], bf16, kind="ExternalOutput")

    GSC = 0.125 / (WSCALE * WSCALE)

    with tile.TileContext(nc) as tc:
        with (
            tc.tile_pool(name="res", bufs=1) as res,
            tc.tile_pool(name="acts", bufs=1) as acts,
            tc.tile_pool(name="work", bufs=2) as work,
            tc.tile_pool(name="outp", bufs=4) as outp,
            tc.tile_pool(name="psum", bufs=1, space="PSUM") as psum,
        ):
            # ---------------- resident input tiles ----------------
            qT = [res.tile([128, S], bf16, tag=f"qT{i}", name=f"qT{i}") for i in range(ND)]
            kT = [res.tile([128, S], bf16, tag=f"kT{i}", name=f"kT{i}") for i in range(ND)]
            vT = [res.tile([128, S], bf16, tag=f"vT{i}", name=f"vT{i}") for i in range(ND)]
            wqp = res.tile([128, ND * C], bf16, tag="wqp")
            wkp = res.tile([128, ND * C], bf16, tag="wkp")
            wvp = res.tile([128, ND * C], bf16, tag="wvp")
            wf = [res.tile([128, D], bf16, tag=f"wf{i}", name=f"wf{i}") for i in range(2)]
            wg4 = [res.tile([128, HPC], bf16, tag=f"wg4{i}", name=f"wg4{i}")
                   for i in range(2)]
            bq = res.tile([1, C], bf16, tag="bq")
            bk = res.tile([1, C], bf16, tag="bk")
            bv = res.tile([1, C], bf16, tag="bv")
            bg128 = res.tile([128, 1], f32, tag="bg128")
            ones = res.tile([1, 512], bf16, tag="ones")

            def wsl(wt, dc, cb):
                return wt[:, dc * C + cb * 128: dc * C + (cb + 1) * 128]

            # ---------------- DMA emission (arrival order = priority) ------
            nc.sync.dma_start(wkp[:], wk_d[:, :])
            nc.sync.dma_start(bk[:], bk_d[:])
            nc.sync.dma_start(bg128[:], bg_d[:])
            for i in range(2):
                nc.sync.dma_start(wg4[i][:], wg4_d[i * 128:(i + 1) * 128, :])
            for cols in (slice(0, 1024), slice(1024, 2048)):   # kT in halves
                for i in range(ND):
                    nc.sync.dma_start(kT[i][:, cols], kT_d[i * 128:(i + 1) * 128, cols])
            nc.sync.dma_start(wqp[:], wq_d[:, :])
            nc.sync.dma_start(bq[:], bq_d[:])
            nc.sync.dma_start(wvp[:], wv_d[:, :])
            nc.sync.dma_start(bv[:], bv_d[:])
            for i in range(ND):   # vT first quarter: V-proj 0..3 fill the
                nc.sync.dma_start(vT[i][:, 0:512],   # prephase PE-idle window
                                  vT_d[i * 128:(i + 1) * 128, 0:512])
            for i in range(ND):   # qT first half (Q proj a=0)
                nc.sync.dma_start(qT[i][:, 0:1024], qT_d[i * 128:(i + 1) * 128, 0:1024])
            for cols in (slice(512, 1024), slice(1024, 2048)):
                for i in range(ND):
                    nc.sync.dma_start(vT[i][:, cols], vT_d[i * 128:(i + 1) * 128, cols])
            for i in range(ND):   # qT second half
                nc.sync.dma_start(qT[i][:, 1024:2048],
                                  qT_d[i * 128:(i + 1) * 128, 1024:2048])
            for i in range(2):
                nc.sync.dma_start(wf[i][:], wf_d[i * 128:(i + 1) * 128, :])
            nc.vector.memset(ones[:], 1.0)

            # ---------------- persistent activation tiles ----------------
            QT = [acts.tile([128, S], bf16, tag=f"QT{i}", name=f"QT{i}") for i in range(2)]
            KT = [acts.tile([128, S], bf16, tag=f"KT{i}", name=f"KT{i}") for i in range(2)]
            ctxT = [acts.tile([128, S], bf16, tag=f"ctxT{i}", name=f"ctxT{i}")
                    for i in range(2)]
            gscT = acts.tile([128, NSB * HPC], f32, tag="gscT")
            V = [acts.tile([128, HPC * 2 * DH], bf16, tag=f"V{i}", name=f"V{i}")
                 for i in range(NSB)]

            st_alt = [0]

            def project_chunk(wt, x_tiles, bias, dest, cb, qs, pname,
                              tag=None, skip_gc=False):
                # one 512-wide output chunk dest[:, qs] (+ rank-1 bias)
                if tag is None:
                    tag = f"st{st_alt[0] % 2}"
                    st_alt[0] += 1
                ps = psum.tile([128, 512], f32, tag=tag, name=pname)
                for dc in range(ND):
                    nc.tensor.matmul(
                        ps[:], wsl(wt, dc, cb), x_tiles[dc][:, qs],
                        start=(dc == 0), stop=False, skip_group_check=skip_gc,
                    )
                nc.tensor.matmul(
                    ps[:], bias[0:1, cb * 128:(cb + 1) * 128], ones[0:1, 0:512],
                    start=False, stop=True, skip_group_check=skip_gc,
                )
                nc.vector.tensor_copy(dest[:, qs], ps[:])

            def emit_vproj(sb, tag):
                nc.vector.memset(V[sb][:], 1.0)
                ps = psum.tile([128, C], f32, tag=tag, name=f"psv{sb}")
                for dc in range(ND):
                    nc.tensor.matmul(
                        ps[:], vT[dc][:, sb * 128:(sb + 1) * 128],
                        wvp[:, dc * C:(dc + 1) * C],
                        start=(dc == 0), stop=False, skip_group_check=True,
                    )
                nc.tensor.matmul(ps[:], ones[0:1, 0:128], bv[:], start=False,
                                 stop=True, skip_group_check=True)
                nc.vector.tensor_copy(
                    V[sb][:].rearrange("p (h x) -> p h x", h=HPC)[:, :, 0:DH],
                    ps[:].rearrange("p (h x) -> p h x", h=HPC),
                )

            def emit_outproj(a, qb, tag, cast_engine=None):
                # out rows [a*1024 + qb*128 : +128]; contract ctx dims (2 cb)
                ps = psum.tile([128, 1024], f32, tag=tag, name=f"pso{a}{qb}")
                row0 = a * 1024 + qb * 128
                for oc in range(2):
                    for cc in range(2):
                        nc.tensor.matmul(
                            ps[:, oc * 512:(oc + 1) * 512],
                            ctxT[cc][:, row0:row0 + 128],
                            wf[cc][:, oc * 512:(oc + 1) * 512],
                            start=(cc == 0), stop=(cc == 1),
                            skip_group_check=True,
                        )
                ob = outp.tile([128, D], bf16, tag="ob", name=f"ob{a}{qb}")
                # split the PSUM-freeing cast across Scalar+Vector so the aux
                # slot releases in ~0.6us even when the DVE queue is backed up
                # behind a normalize reciprocal
                nc.scalar.activation(ob[:, 0:512], ps[:, 0:512], Act.Copy)
                nc.vector.tensor_copy(ob[:, 512:1024], ps[:, 512:1024])
                nc.sync.dma_start(out_d[row0:row0 + 128, :], ob[:])

            # ---------------- prephase: K proj + gates pipelined with DMA --
            gps = psum.tile([128, NSB * HPC], f32, tag="aux0", name="gps")
            gn = work.tile([128, NSB * HPC], f32, tag="gn", bufs=1)
            for t in range(4):
                ts_ = slice(t * 512, (t + 1) * 512)
                for cb in range(2):
                    project_chunk(wkp, kT, bk, KT[cb], cb, ts_, f"psk{cb}{t}")
                for sb in range(4 * t, 4 * t + 4):
                    for cc in range(2):
                        nc.tensor.matmul(
                            gps[:, sb * HPC:(sb + 1) * HPC],
                            KT[cc][:, sb * 128:(sb + 1) * 128],
                            wg4[cc][:],
                            start=(cc == 0), stop=(cc == 1),
                        )
                gsl = slice(4 * t * HPC, (4 * t + 4) * HPC)
                nc.scalar.activation(gn[:, gsl], gps[:, gsl], Act.Sigmoid,
                                     bias=bg128[:], scale=1.0)
                nc.vector.tensor_scalar(gscT[:, gsl], gn[:, gsl], GSC, GSC,
                                        Alu.mult, Alu.add)

            # V proj 0..3 in the DMA-wait window, then Q proj cb0-a0
            for sb in range(4):
                emit_vproj(sb, "aux1")
            for t in range(2):
                project_chunk(wqp, qT, bq, QT[0], 0, slice(t * 512, (t + 1) * 512),
                              f"psq00{t}")

            # ---------------- pop schedule ----------------
            pops = {}

            def add_pop(si, kb, fn):
                pops.setdefault((si, kb), []).append(fn)

            # V proj 4..14 during head 0 (its PV(kb>=8) defer to head 1)
            for sb in range(4, NSB - 1):
                add_pop(0, sb - 3, (lambda sb=sb: emit_vproj(sb, "aux1")))
            def qproj_job(cb, ah, tag):
                # both 512-chunks of QT[cb] half `ah` in one psum tile with a
                # single PSUM-freeing cast (two casts would serialize the
                # next job's matmuls behind the DVE queue)
                ps = psum.tile([128, 1024], f32, tag=tag, name=f"psq{cb}{ah}")
                for t in range(2):
                    ph = ps[:, t * 512:(t + 1) * 512]
                    qs = slice(ah * 1024 + t * 512, ah * 1024 + (t + 1) * 512)
                    for dc in range(ND):
                        nc.tensor.matmul(
                            ph, wsl(wqp, dc, cb), qT[dc][:, qs],
                            start=(dc == 0), stop=False, skip_group_check=True)
                    nc.tensor.matmul(
                        ph, bq[0:1, cb * 128:(cb + 1) * 128], ones[0:1, 0:512],
                        start=False, stop=True, skip_group_check=True)
                nc.vector.tensor_copy(
                    QT[cb][:, ah * 1024:(ah + 1) * 1024], ps[:])

            # Q proj cb1 a0 (needed at si=2): aux0 freed by otc(0,0) @ (1,kb9)
            add_pop(1, 11, lambda: qproj_job(1, 0, "aux0"))
            # Q proj a1 (needed at si=4/si=6): aux1 freed by otc(0,1) @ (2,kb1)
            add_pop(2, 3, lambda: qproj_job(0, 1, "aux1"))
            add_pop(2, 8, lambda: qproj_job(1, 1, "aux1"))
            # out-proj of half 0: after normfin(0,3) completes ~ (4,kb8)
            for j in range(4):
                add_pop(4, 9 + 2 * j, (lambda j=j: emit_outproj(0, j, "aux1")))
            for j in range(4, 8):
                add_pop(5, 2 + 3 * (j - 4),
                        (lambda j=j: emit_outproj(0, j, "aux0")))

            # ---------------- attention: continuous cross-head pipeline ----
            seq = [(a, h) for a in range(NA) for h in range(HPC)]
            # normfin(si) pop slots: (target_si, [kb_c0, kb_c1])
            NF_SLOT = {0: (2, (2, 4)), 1: (3, (2, 4)), 2: (3, (6, 8)),
                       3: (4, (2, 4)), 4: (6, (2, 4)), 5: (7, (2, 4)),
                       6: (7, (6, 8))}
            carry = []   # closures from the previous head, one per early iter

            for si, (a, h) in enumerate(seq):
                qs = slice(a * AW, (a + 1) * AW)
                cb, po = h // 2, (h % 2) * 64
                vsl = slice(h * 2 * DH, (h + 1) * 2 * DH)
                ot = psum.tile([128, AW], f32, tag=f"aux{h % 2}",
                               name=f"ot{a}{h}")
                defer = 8 if si == 0 else 15   # PV(kb>=defer) carried onward
                pts = {}

                def mk_pv(j, ot=ot, vsl=vsl, pts=pts, defer=defer):
                    def f():
                        ptp = pts.pop(j)
                        for qc in range(2):
                            cs = slice(qc * 512, (qc + 1) * 512)
                            nc.tensor.matmul(
                                ot[:, cs], V[j][:, vsl], ptp[:, cs],
                                start=(j == 0), stop=(j == NSB - 1),
                                skip_group_check=True)
                    return f

                for kb in range(NSB):
                    kslc = slice(kb * 128, (kb + 1) * 128)
                    st = psum.tile([128, AW], f32, tag=f"st{kb % 2}",
                                   name=f"st{a}{h}{kb}")
                    for qc in range(2):
                        cs = slice(qc * 512, (qc + 1) * 512)
                        aqs = slice(a * AW + qc * 512, a * AW + (qc + 1) * 512)
                        nc.tensor.matmul(st[:, cs], KT[cb][po:po + 64, kslc],
                                         QT[cb][po:po + 64, aqs],
                                         start=True, stop=True,
                                         skip_group_check=True)
                    pt = work.tile([128, AW], bf16, tag="pt",
                                   name=f"pt{a}{h}{kb}", bufs=10)
                    nc.scalar.activation(pt[:], st[:], Act.Exp,
                                         scale=gscT[:, kb * HPC + h:kb * HPC + h + 1])
                    pts[kb] = pt
                    if carry:
                        carry.pop(0)()
                    for fn in pops.get((si, kb), ()):
                        fn()
                    if 1 <= kb and kb - 1 < defer:
                        mk_pv(kb - 1)()

                # hand the rest of this head to the next heads' iterations
                newcarry = [mk_pv(j) for j in range(defer, NSB)]
                if si == 0:
                    newcarry.insert(0, lambda: emit_vproj(NSB - 1, "aux1"))

                def mk_otc(si=si, a=a, h=h, cb=cb, po=po, ot=ot):
                    otc = work.tile([128, AW], f32, tag="otc",
                                    name=f"otc{a}{h}", bufs=2)
                    nc.vector.tensor_copy(otc[:], ot[:])

                    def norm_fin_chunk(part):
                        rec = work.tile([DH, 512], f32, tag=f"rec{part}",
                                        name=f"rec{a}{h}{part}", bufs=2)
                        cs = slice(part * 512, (part + 1) * 512)
                        qcs = slice(a * AW + part * 512, a * AW + (part + 1) * 512)
                        with tc.high_priority(offset=-256):
                            nc.vector.reciprocal(rec[:], otc[DH:2 * DH, cs])
                            nc.vector.tensor_tensor(
                                ctxT[cb][po:po + 64, qcs], otc[0:DH, cs],
                                rec[:], Alu.mult)

                    if si in NF_SLOT:
                        tsi, kbs = NF_SLOT[si]
                        add_pop(tsi, kbs[0], lambda: norm_fin_chunk(0))
                        add_pop(tsi, kbs[1], lambda: norm_fin_chunk(1))
                        return None
                    return otc

                if si + 1 < len(seq):
                    newcarry.append(mk_otc)
                    carry = newcarry
                else:
                    # tail: drain the last PV, then normalize on the idle ACT
                    # engine: exp(-ln(den)), chunked mult + out-proj of half 1.
                    for c in newcarry:
                        c()
                    otcf = mk_otc()
                    lt = work.tile([DH, AW], f32, tag="ltf", name="lt", bufs=1)
                    rec = work.tile([DH, AW], f32, tag="recf", name="recf", bufs=1)
                    nc.scalar.activation(lt[:], otcf[DH:2 * DH, :], Act.Ln)
                    nc.scalar.activation(rec[:], lt[:], Act.Exp, scale=-1.0)
                    for cchunk in range(4):
                        cs = slice(cchunk * 256, (cchunk + 1) * 256)
                        qcs = slice(a * AW + cchunk * 256,
                                    a * AW + (cchunk + 1) * 256)
                        nc.vector.tensor_tensor(
                            ctxT[cb][po:po + 64, qcs], otcf[0:DH, cs],
                            rec[:, cs], Alu.mult)
                        for qb in (2 * cchunk, 2 * cchunk + 1):
                            emit_outproj(1, qb, f"aux{qb % 2}",
                                         cast_engine=("scalar", None)[qb % 2])

    nc.finalize()
    return nc


def get_nc():
    if "nc" not in _nc_cache:
        _nc_cache["nc"] = build_bass()
    return _nc_cache["nc"]


def pack_w(W, cols):
    """[D, C] weight slice -> packed [128, ND*C] (dc-major along free)."""
    Wc = np.ascontiguousarray(np.asarray(W, np.float32)[:, cols])
    return np.concatenate([Wc[dc * 128:(dc + 1) * 128, :] for dc in range(ND)],
                          axis=1).astype(BF16)


def make_in_maps(query, key_, value, Wq, bq, Wk, bk, Wv, bv, wg, bg, Wo, bo, Wd, bd, Wh, bh):
    """Host-side sharding: returns (in_maps for 8 cores, fused bias)."""
    f = np.asarray
    Wf = f(Wo, np.float64) @ f(Wd, np.float64) @ f(Wh, np.float64)
    bf = (f(bo, np.float64) @ f(Wd, np.float64) @ f(Wh, np.float64)
          + f(bd, np.float64) @ f(Wh, np.float64) + f(bh, np.float64))

    wg4 = np.zeros((C, HPC), np.float32)
    for h in range(HPC):
        wg4[h * DH:(h + 1) * DH, h] = np.asarray(wg, np.float32)
    wg4 = wg4.astype(BF16)
    bg128 = np.full((128, 1), np.float32(bg), np.float32)

    xT = []
    for b in range(B):
        xT.append(tuple(
            np.ascontiguousarray(np.asarray(x[b], np.float32).T).astype(BF16)
            for x in (query, key_, value)
        ))

    in_maps = []
    for c in range(NCORES):
        b, g = divmod(c, HPC)
        cols = slice(g * C, (g + 1) * C)
        qTb, kTb, vTb = xT[b]
        in_maps.append({
            "qT": qTb, "kT": kTb, "vT": vTb,
            "wq": pack_w(Wq, cols), "wk": pack_w(Wk, cols), "wv": pack_w(Wv, cols),
            "wf": np.ascontiguousarray(Wf[cols, :]).astype(BF16),
            "wg4": wg4, "bg128": bg128,
            "bq": np.asarray(bq, np.float32)[None, cols].astype(BF16),
            "bk": np.asarray(bk, np.float32)[None, cols].astype(BF16),
            "bv": np.asarray(bv, np.float32)[None, cols].astype(BF16),
        })
    return in_maps, bf.astype(np.float32)


def gather(results, bf):
    out = np.zeros((B, S, D), np.float32)
    for c in range(NCORES):
        b = c // HPC
        out[b] += np.asarray(results[c]["out"], np.float32)
    out += bf[None, None, :]
    return out


def kernel(**inputs):
    from concourse.bass_utils import run_bass_kernel_spmd

    nc = get_nc()
    in_maps, bf = make_in_maps(**inputs)
    res = run_bass_kernel_spmd(nc, in_maps, core_ids=list(range(NCORES)))
    return gather(res.results, bf)
